# revision 50
# baseline (speedup 1.0000x reference)
"""Trainium2 Bass kernel for nn_AdapterBlock (LN -> dwconv x3 -> SE -> residual).

Data-parallel over batch: 8 samples -> 8 NeuronCores. Per core:
  - cast x f32->bf16 while DMA'ing into SBUF layout A [128 t_lo, 16 t_hi, 1024 c]
  - LayerNorm stats (ScalarE accum passes / DVE bn_stats), fused apply
  - xbar DMA-transpose to layout B via contiguous stage + engine re-scatter
  - conv1 (k=3): diagonal-matmul PSUM accumulation on TensorE (or DVE MACs),
    relu + folded ln_b bias fused into the ScalarE PSUM evacuation
  - conv2*conv3 fused into one k=7 depthwise conv on TensorE (host-composed
    weights); SAME-boundary mismatch fixed by 2 extra accumulate-matmuls per
    edge; SE global-average pool rides the evacuation's accum_out
  - SE MLP on TensorE, sigmoid gate on ScalarE, per-partition gate scale
  - xbar transpose back into a contiguous stage, strided-read residual add,
    cast-store f32
"""

import os
import sys

sys.path.insert(0, "/opt/trn_rl_repo")

from contextlib import ExitStack

import numpy as np

import concourse.bass as bass  # noqa: F401
import concourse.bacc as bacc
import concourse.tile as tile
import concourse.mybir as mybir
from concourse.bass_utils import run_bass_kernel_spmd

B, T, C = 8, 2048, 1024
N_CORES = 8
NT = T // 128          # 16 t-tiles
NCH = C // 128         # 8 channel groups
H = C // 16            # SE hidden = 64
PAD = 4                # zero pad each side of the time axis (>= conv halo 3)
TF = T + 2 * PAD
HT = T // 2            # half-tile free size for PSUM (2 banks)
EPS = 1e-5

F32 = mybir.dt.float32
BF16 = mybir.dt.bfloat16
AF = mybir.ActivationFunctionType
OP = mybir.AluOpType

# --- tunables ------------------------------------------------------------
CONV1_PE_MASK = int(os.environ.get("K_CONV1_PE_MASK", "0"), 0)
N_STATS_ACT = int(os.environ.get("K_STATS_ACT", "0"))   # first N t-tiles on ScalarE
ADDS_POOL = int(os.environ.get("K_ADDS_POOL", "0"))     # of 16 residual adds
RESCATTER_ACT = int(os.environ.get("K_RESCATTER_ACT", "8"))  # of 16 copies
K_DEBUG = int(os.environ.get("K_DEBUG", "0"))

_CACHE = {}


def _build():
    nc = bacc.Bacc("TRN2", target_bir_lowering=False, debug=False,
                   num_devices=N_CORES)

    x_ext = nc.dram_tensor("x", [T, C], F32, kind="ExternalInput").ap()
    res_ext = nc.dram_tensor("res", [T, C], F32, kind="ExternalInput").ap()
    w1_ext = nc.dram_tensor("w1p", [128, NCH, 3], F32, kind="ExternalInput").ap()
    b1_ext = nc.dram_tensor("b1p", [128, NCH], F32, kind="ExternalInput").ap()
    fc1_ext = nc.dram_tensor("fc1p", [128, NCH, H], F32, kind="ExternalInput").ap()
    fc2_ext = nc.dram_tensor("fc2p", [H, NCH, 128], F32, kind="ExternalInput").ap()
    # diag weight stacks (pre-swizzled on host to partition-major layout)
    d1_ext = nc.dram_tensor("d1", [128, 3, NCH, 128], BF16, kind="ExternalInput").ap()
    d23_ext = nc.dram_tensor("d23", [128, 7, NCH, 128], BF16, kind="ExternalInput").ap()
    dec_ext = nc.dram_tensor("dec", [128, 4, NCH, 128], BF16, kind="ExternalInput").ap()
    out_ext = nc.dram_tensor("out", [T, C], F32, kind="ExternalOutput").ap()

    if K_DEBUG:
        dbg_xb_ext = nc.dram_tensor("d_xb", [128, NCH, TF], F32,
                                    kind="ExternalOutput").ap()
        dbg_r_ext = nc.dram_tensor("d_r", [128, NCH, TF], F32,
                                   kind="ExternalOutput").ap()
        dbg_c3_ext = nc.dram_tensor("d_c3", [128, NCH, TF], F32,
                                    kind="ExternalOutput").ap()
        dbg_pg_ext = nc.dram_tensor("d_pg", [128, NCH, 4], F32,
                                    kind="ExternalOutput").ap()

    x_src = x_ext.rearrange("(th p) c -> p th c", p=128)
    res_src = res_ext.rearrange("(th p) c -> p th c", p=128)
    out_dst = out_ext.rearrange("(th p) c -> p th c", p=128)

    with tile.TileContext(nc) as tc, ExitStack() as ctx:
        pool = ctx.enter_context(tc.tile_pool(name="main", bufs=1))

        # ---- weights in ----
        w1sb = pool.tile([128, NCH, 3], F32, tag="w1sb")
        b1sb = pool.tile([128, NCH], F32, tag="b1sb")
        fc1sb = pool.tile([128, NCH, H], F32, tag="fc1sb")
        fc2sb = pool.tile([H, NCH, 128], F32, tag="fc2sb")
        d1sb = pool.tile([128, 3, NCH, 128], BF16, tag="d1sb")
        d23sb = pool.tile([128, 7, NCH, 128], BF16, tag="d23sb")
        decsb = pool.tile([128, 4, NCH, 128], BF16, tag="decsb")
        nc.sync.dma_start(w1sb[:], w1_ext)
        nc.sync.dma_start(b1sb[:], b1_ext)
        nc.sync.dma_start(fc1sb[:], fc1_ext)
        nc.sync.dma_start(fc2sb[:], fc2_ext)
        nc.sync.dma_start(d1sb[:], d1_ext)
        nc.sync.dma_start(d23sb[:], d23_ext)
        nc.sync.dma_start(decsb[:], dec_ext)

        # ---- phases 1-5, pipelined per 4-t-tile group:
        #   cast-load -> LN stats -> mu/rstd -> LN apply -> xbar -> scatter
        from concourse.tile_rust import add_dep_helper
        NA = N_STATS_ACT
        zX = pool.tile([128, NT, C], BF16, tag="zX")
        sums = pool.tile([128, NT], F32, tag="sums")
        sumsq = pool.tile([128, NT], F32, tag="sumsq")
        scr = pool.tile([128, C], BF16, tag="scr")
        scr2 = pool.tile([128, C], BF16, tag="scr2")
        mu = pool.tile([128, NT], F32, tag="mu")
        rstd = pool.tile([128, NT], F32, tag="rstd")
        varv = pool.tile([128, NT], F32, tag="varv")
        epsb = pool.tile([128, 1], F32, tag="epsb")
        nc.vector.memset(epsb[:], EPS)
        stage = pool.tile([128, NT * C], BF16, tag="stage")
        stg_ab = stage[:].rearrange("p (th ch t) -> p th ch t", th=NT, ch=NCH)
        xB = pool.tile([128, NCH, TF], BF16, tag="xB")
        nc.vector.memset(xB[:, :, 0:PAD], 0.0)
        nc.vector.memset(xB[:, :, PAD + T:TF], 0.0)

        for g in range(4):
            ts0 = 4 * g
            nc.gpsimd.dma_start(zX[:, ts0:ts0 + 2, :],
                                x_src[:, ts0:ts0 + 2, :])
            nc.gpsimd.dma_start(zX[:, ts0 + 2:ts0 + 4, :],
                                x_src[:, ts0 + 2:ts0 + 4, :])
            for t in range(ts0, ts0 + 4):
                if t < NA:
                    nc.scalar.activation(scr[:], zX[:, t, :], AF.Copy,
                                         accum_out=sums[:, t:t + 1])
                    nc.scalar.activation(scr2[:], zX[:, t, :], AF.Square,
                                         accum_out=sumsq[:, t:t + 1])
                else:
                    bs = pool.tile([128, 2, 6], F32, tag="bstats",
                                   name=f"bs_{t}", bufs=2)
                    nc.vector.bn_stats(bs[:, 0, :], zX[:, t, 0:512])
                    nc.vector.bn_stats(bs[:, 1, :], zX[:, t, 512:1024])
                    agg = pool.tile([128, 2], F32, tag="agg",
                                    name=f"agg_{t}", bufs=2)
                    nc.vector.bn_aggr(agg[:], bs[:])
                    nc.vector.tensor_copy(mu[:, t:t + 1], agg[:, 0:1])
                    nc.vector.tensor_copy(varv[:, t:t + 1], agg[:, 1:2])
            gs = slice(ts0, ts0 + 4)
            if ts0 < NA:  # ACT-stats tiles in this group
                nc.vector.tensor_scalar_mul(mu[:, gs], sums[:, gs], 1.0 / C)
                nc.vector.tensor_tensor(varv[:, gs], mu[:, gs], mu[:, gs],
                                        op=OP.mult)
                nc.vector.scalar_tensor_tensor(varv[:, gs], sumsq[:, gs],
                                               1.0 / C, varv[:, gs],
                                               OP.mult, OP.subtract)
            nc.scalar.activation(varv[:, gs], varv[:, gs], AF.Sqrt,
                                 bias=epsb[:])
            nc.vector.reciprocal(rstd[:, gs], varv[:, gs])
            for t in range(ts0, ts0 + 4):
                nc.vector.tensor_scalar(zX[:, t, :], zX[:, t, :],
                                        mu[:, t:t + 1], rstd[:, t:t + 1],
                                        OP.subtract, OP.mult)
            # xbar this group (one 1MB call), scatter into conv layout
            tr = nc.sync.dma_start(
                out=stg_ab[:, ts0:ts0 + 4, :, :],
                in_=zX[:, ts0:ts0 + 4, :].rearrange("p a b -> p (a b)"),
                transpose=True)
            for t in range(ts0, ts0 + 4):
                dst = xB[:, :, PAD + t * 128:PAD + (t + 1) * 128]
                cp = nc.vector.tensor_copy(dst, stg_ab[:, t, :, :])
                add_dep_helper(cp.ins, tr.ins, reason="xbar ordering")

        # residual in: reuse zX (consumed by the transposes above); overlaps
        # the conv phase entirely.
        for q in range(4):
            nc.gpsimd.dma_start(zX[:, q * 4:(q + 1) * 4, :],
                                res_src[:, q * 4:(q + 1) * 4, :])

        # all of xB is needed by every conv tile anyway; the barrier makes
        # every later DMA-lane wait a final-threshold wait (order-safe).
        if int(os.environ.get("K_BARRIERS", "0")):
            tc.strict_bb_all_engine_barrier()

        # ---- phase 6: convs ----
        rr = [pool.tile([128, TF], BF16, tag=f"rr_{i}", name=f"rr_{i}")
              for i in range(2)]
        acc = [pool.tile([128, T], BF16, tag=f"acc_{i}", name=f"acc_{i}")
               for i in range(2)]
        tmpc = [pool.tile([128, T], BF16, tag=f"tmpc_{i}", name=f"tmpc_{i}")
                for i in range(2)]
        for i in range(2):
            nc.vector.memset(rr[i][:, 0:PAD], 0.0)
            nc.vector.memset(rr[i][:, PAD + T:TF], 0.0)
        c3 = pool.tile([128, NCH, TF], BF16, tag="c3")
        nc.vector.memset(c3[:, :, 0:PAD], 0.0)
        nc.vector.memset(c3[:, :, PAD + T:TF], 0.0)
        pools = pool.tile([128, NCH, 2], F32, tag="pools")

        psum = ctx.enter_context(tc.tile_pool(name="ps", bufs=3, space="PSUM"))

        for ch in range(NCH):
            r = rr[ch % 2]

            def xs(d):
                return xB[:, ch, PAD + d:PAD + d + T]

            # conv1: k=3, shifts -1..1
            if (CONV1_PE_MASK >> ch) & 1:
                for hh in range(2):
                    base = hh * HT
                    ps1 = psum.tile([128, HT], F32, tag="cps",
                                    name=f"c1ps_{ch}_{hh}")
                    for k in range(3):
                        for q in range(2):
                            off = PAD - 1 + k + base + q * 512
                            nc.tensor.matmul(ps1[:, q * 512:(q + 1) * 512],
                                             d1sb[:, k, ch, :],
                                             xB[:, ch, off:off + 512],
                                             start=(k == 0), stop=(k == 2))
                    nc.scalar.activation(r[:, PAD + base:PAD + base + HT],
                                         ps1[:], AF.Relu,
                                         bias=b1sb[:, ch:ch + 1])
            else:
                a = acc[ch % 2]
                tm = tmpc[ch % 2]
                # two halves (h0 extended past the seam by the conv23 halo)
                seam = HT + 4
                for lo, hi in ((0, seam), (seam, T)):
                    w = slice(lo, hi)
                    def xsw(d, lo=lo, hi=hi):
                        return xB[:, ch, PAD + lo + d:PAD + hi + d]
                    nc.vector.tensor_scalar(a[:, w], xsw(-1), w1sb[:, ch, 0:1],
                                            None, OP.mult)
                    nc.vector.tensor_scalar(tm[:, w], xsw(0), w1sb[:, ch, 1:2],
                                            None, OP.mult)
                    nc.vector.tensor_tensor(a[:, w], a[:, w], tm[:, w],
                                            op=OP.add)
                    nc.vector.tensor_scalar(tm[:, w], xsw(1), w1sb[:, ch, 2:3],
                                            None, OP.mult)
                    nc.vector.tensor_tensor(a[:, w], a[:, w], tm[:, w],
                                            op=OP.add)
                    nc.vector.tensor_scalar(r[:, PAD + lo:PAD + hi], a[:, w],
                                            b1sb[:, ch:ch + 1], 0.0,
                                            OP.add, OP.max)

            # fused conv23: k=7, shifts -3..3, plus SAME-boundary fixes
            for hh in range(2):
                base = hh * HT
                ps2 = psum.tile([128, HT], F32, tag="cps",
                                name=f"c23ps_{ch}_{hh}")
                for k in range(7):
                    for q in range(2):
                        off = PAD - 3 + k + base + q * 512
                        edge_q = (hh == 0 and q == 0) or (hh == 1 and q == 1)
                        nc.tensor.matmul(ps2[:, q * 512:(q + 1) * 512],
                                         d23sb[:, k, ch, :],
                                         r[:, off:off + 512],
                                         start=(k == 0),
                                         stop=(k == 6 and not edge_q))
                if hh == 0:
                    # out[0] -= w3[0]*(w2[3] r[0] + w2[4] r[1])
                    nc.tensor.matmul(ps2[:, 0:1], decsb[:, 0, ch, :],
                                     r[:, PAD:PAD + 1], start=False, stop=False)
                    nc.tensor.matmul(ps2[:, 0:1], decsb[:, 1, ch, :],
                                     r[:, PAD + 1:PAD + 2], start=False,
                                     stop=True)
                else:
                    # out[T-1] -= w3[2]*(w2[0] r[T-2] + w2[1] r[T-1])
                    nc.tensor.matmul(ps2[:, HT - 1:HT], decsb[:, 2, ch, :],
                                     r[:, PAD + T - 2:PAD + T - 1],
                                     start=False, stop=False)
                    nc.tensor.matmul(ps2[:, HT - 1:HT], decsb[:, 3, ch, :],
                                     r[:, PAD + T - 1:PAD + T],
                                     start=False, stop=True)
                nc.scalar.activation(c3[:, ch, PAD + base:PAD + base + HT],
                                     ps2[:], AF.Copy,
                                     accum_out=pools[:, ch, hh:hh + 1])
            if K_DEBUG:
                nc.gpsimd.dma_start(dbg_r_ext[:, ch, :], rr[ch % 2][:])

        # ---- phase 7: SE MLP ----
        se_ps = ctx.enter_context(tc.tile_pool(name="seps", bufs=1,
                                               space="PSUM"))
        h_ps = se_ps.tile([H, 2], F32, tag="hps")
        for ch in range(NCH):
            nc.tensor.matmul(h_ps[:], fc1sb[:, ch, :], pools[:, ch, :],
                             start=(ch == 0), stop=(ch == NCH - 1))
        h_half = pool.tile([H, 2], F32, tag="h_half")
        nc.scalar.activation(h_half[:], h_ps[:], AF.Relu)
        h_sb = pool.tile([H, 1], F32, tag="hsb")
        nc.vector.tensor_tensor(h_sb[:], h_half[:, 0:1], h_half[:, 1:2],
                                op=OP.add)
        g_ps = se_ps.tile([128, NCH], F32, tag="gps")
        for ch in range(NCH):
            nc.tensor.matmul(g_ps[:, ch:ch + 1], fc2sb[:, ch, :], h_sb[:],
                             start=True, stop=True)
        gate = pool.tile([128, NCH], F32, tag="gate")
        nc.scalar.activation(gate[:], g_ps[:], AF.Sigmoid)

        if K_DEBUG:
            nc.gpsimd.dma_start(dbg_xb_ext[:], xB[:])
            nc.gpsimd.dma_start(dbg_c3_ext[:], c3[:])
            nc.sync.dma_start(dbg_pg_ext[:, :, 0:2], pools[:])
            nc.sync.dma_start(dbg_pg_ext[:, :, 2], gate[:])

        # ---- phase 8: SE scale (in place) ----
        for ch in range(NCH):
            nc.vector.tensor_scalar(c3[:, ch, :], c3[:, ch, :],
                                    gate[:, ch:ch + 1], None, OP.mult)

        # ---- phase 9: transpose B->A into the (now free) stage ----
        stg_ba = stage[:].rearrange("p (ch th c) -> p ch th c", ch=NCH, th=NT)
        # single ring: dual-ring xbar completions proved unsafe here on HW
        for ch in range(NCH):
            nc.sync.dma_start(out=stg_ba[:, ch, :, :],
                              in_=c3[:, ch, PAD:PAD + T], transpose=True)

        # ---- phase 10: residual add (pair-batched, in place) + store ----
        for q in range(8):
            zt = zX[:, 2 * q:2 * q + 2, :].rearrange(
                "p th (ch c) -> p ch th c", ch=NCH)
            st = stg_ba[:, :, 2 * q:2 * q + 2, :]
            eng = nc.gpsimd if (ADDS_POOL and q % 4 == 3) else nc.vector
            eng.tensor_tensor(zt, zt, st, op=OP.add)
            nc.gpsimd.dma_start(out_dst[:, 2 * q:2 * q + 2, :],
                                zX[:, 2 * q:2 * q + 2, :])

    nc.compile()
    return nc


def _prep_weights(ln_w, ln_b, w1, w2, w3, fc1, fc2):
    import ml_dtypes
    w1 = w1[:, 0, :].astype(np.float64)   # [C, 3]
    w2 = w2[:, 0, :].astype(np.float64)   # [C, 5]
    w3 = w3[:, 0, :].astype(np.float64)   # [C, 3]
    ln_w = ln_w.astype(np.float64)
    ln_b = ln_b.astype(np.float64)
    w1f = w1 * ln_w[:, None]
    b1 = (ln_b * w1.sum(axis=1))

    def to_plh(a):  # [C, K] -> [128, NCH, K]
        return np.ascontiguousarray(
            a.reshape(NCH, 128, -1).transpose(1, 0, 2)).astype(np.float32)

    w1p = to_plh(w1f)
    b1p = np.ascontiguousarray(b1.reshape(NCH, 128).T).astype(np.float32)
    fc1p = to_plh((fc1.astype(np.float64) / T).T)
    fc2p = np.ascontiguousarray(
        fc2.astype(np.float64).T.reshape(H, NCH, 128)).astype(np.float32)

    w23 = np.stack([np.convolve(w3[c], w2[c]) for c in range(C)])  # [C, 7]
    # edge-fix coefficients (negated: they accumulate into the psum)
    ec = np.stack([-w3[:, 0] * w2[:, 3], -w3[:, 0] * w2[:, 4],
                   -w3[:, 2] * w2[:, 0], -w3[:, 2] * w2[:, 1]], axis=1)  # [C,4]

    def diags(wk):  # [C, K] -> [128, K, NCH, 128] bf16 (partition-major)
        K = wk.shape[1]
        d = np.zeros((K, NCH, 128, 128), np.float32)
        for k in range(K):
            for chh in range(NCH):
                np.fill_diagonal(d[k, chh], wk[chh * 128:(chh + 1) * 128, k])
        return np.ascontiguousarray(
            d.transpose(2, 0, 1, 3)).astype(ml_dtypes.bfloat16)

    return {"w1p": w1p, "b1p": b1p, "fc1p": fc1p, "fc2p": fc2p,
            "d1": diags(w1f), "d23": diags(w23), "dec": diags(ec)}


def kernel(x, residual_input, ln_w, ln_b, w1, w2, w3, fc1, fc2):
    x = np.asarray(x, dtype=np.float32)
    residual_input = np.asarray(residual_input, dtype=np.float32)
    wts = _prep_weights(np.asarray(ln_w), np.asarray(ln_b),
                        np.asarray(w1), np.asarray(w2), np.asarray(w3),
                        np.asarray(fc1), np.asarray(fc2))

    if "nc" not in _CACHE:
        _CACHE["nc"] = _build()
    nc = _CACHE["nc"]

    in_maps = []
    for b in range(B):
        m = {"x": np.ascontiguousarray(x[b]),
             "res": np.ascontiguousarray(residual_input[b])}
        m.update(wts)
        in_maps.append(m)
    res = run_bass_kernel_spmd(nc, in_maps, core_ids=list(range(N_CORES)))
    out = np.stack([res.results[i]["out"] for i in range(N_CORES)], axis=0)
    return out.astype(np.float32)
